# revision 8
# baseline (speedup 1.0000x reference)
"""Trainium2 Bass kernel for the CIN-style layer:

    z   = einsum('btf,byf->bfty', x_0, x_k)            # pairwise outer products
    z   = z.reshape(bs, ts0, f, tsk)                   # flat reinterpretation
    out = einsum('btiy,nty->bni', z, conv_w) + conv_b  # strided conv reduction

Shapes: x_0 (32, 64, 256), x_k (32, 64, 256), conv_w (128, 64, 64),
conv_b (128,) -> out (32, 128, 256).

Math: with i = a*64 + m  (a = i//64, m = i%64) and feature f = 4t + a the
reference reduces to a two-step factorization:

    W2[b,n,t,a]      = sum_y x_k[b,y,4t+a] * conv_w[n,t,y]         (contract y)
    out[b,n,a*64+m]  = sum_t x_0[b,m,4t+a] * W2[b,n,t,a] + conv_b  (contract t)

Sharding: pure data parallel over batch, 4 samples per core, conv_w/conv_b
replicated (no collectives).

Device mapping (v13, all-bf16 compute + bf16 output, fp32 PSUM).  Per core,
c = 4*b_loc + a in [0,16), c = 2*c2 + h, t = 2p + q' with pair p in [0,32):

  step 1 (n-stationary): 32 matmuls, one per t-pair p:
      lhsT = WT[:, 128p:+128]      [K=128 (q,y), M=128 (n)]  (bf16)
      rhs  = XKpad[:, 32p:+32]     [K=128 (q,y), N=32 (q',c)] (bf16,
             zero-padded block-diagonal in q==q', SHIPPED padded from host)
      -> PSUM [n; 32p+16q'+2c2+h] = W2[c, n, t=2p+q']
  scatter copy (DVE+ACT, one per (u-half, h)): PSUM -> SBUF bf16 in the
      shuffle layout  W2n[n; 128c2 + 64h + t].
  shuffle: 8 plain matmuls against a shipped bf16 identity:
      lhsT = W2n[:, 128c2:+128], rhs = I  ->  PSUM [(64h+t); n]; cast
      copies -> w2r bf16.
  step 2: per c-pair c2, PSUM pre-loaded with the bias via a K=1 matmul
      (ones-row (x) bias-row), then
      lhsT = X0pad[:, 128c2:+128]  [K=128 (64h+t), M=128 (h',m)]  (bf16,
             block-diagonal in h==h', SHIPPED padded from host)
      rhs  = w2r[:, 128c2:+128]    accumulated on top (start=False)
      -> PSUM [64h+m, n] = out + bias, cast-copied to bf16 SBUF, DMA'd out.

DMA (v13): weights split across BOTH HWDGE rings (sync: chunks 0-1,
scalar: chunks 2-3, 512 KB each as ONE dma_start per ring, 4 KB/partition
descriptors); xk_pad+identity, x0_pad and the bias ride the gpsimd SWDGE
ring.  Output is bf16 (host upcasts), halves on sync/scalar.  No on-chip
padding glue: only two small memsets (warm-up tile, ones row) remain.
"""

import numpy as np
import ml_dtypes

BS, TS, F, NF = 32, 64, 256, 128
NCORES = 8
B = BS // NCORES  # 4 local batches per core

F32 = np.float32
BF16 = ml_dtypes.bfloat16


# ---------------------------------------------------------------------------
# Host-side packing
# ---------------------------------------------------------------------------

def _pack_wt(conv_w: np.ndarray) -> np.ndarray:
    # WT[64q+y, 128p+n] = conv_w[n, 2p+q, y]
    wt = conv_w.transpose(1, 2, 0).reshape(32, 2, 64, NF)  # [p, q, y, n]
    wt = wt.transpose(1, 2, 0, 3)                          # [q, y, p, n]
    return np.ascontiguousarray(
        wt.reshape(128, 32 * NF).astype(BF16))


def _pack_xk_padded(xk_shard: np.ndarray) -> np.ndarray:
    # XKpad[64q+y, 32p+16q'+c] = (q == q') * xk[b, y, 8p+4q+a],  c = 4b+a
    xq = xk_shard.reshape(B, TS, 32, 2, 4)       # [b, y, p, q, a]
    src = xq.transpose(3, 1, 2, 0, 4)            # [q, y, p, b, a]
    dense = src.reshape(2, TS, 32, 16)           # [q, y, p, c]
    pad = np.zeros((2, TS, 32, 2, 16), dtype=F32)  # [q, y, p, q', c]
    for q in range(2):
        pad[q, :, :, q, :] = dense[q]
    return np.ascontiguousarray(pad.reshape(128, 1024).astype(BF16))


def _pack_x0_padded(x0_shard: np.ndarray) -> np.ndarray:
    # X0pad[64h+t, 128c2+64h'+m] = (h'==h) * x0[b(2c2+h), m, 4t+a(2c2+h)]
    xt = x0_shard.reshape(B, TS, TS, 4).transpose(0, 3, 2, 1)  # [b, a, t, m]
    flat = xt.reshape(16, TS, TS)                              # [c, t, m]
    pad = np.zeros((2, TS, 8, 2, TS), dtype=F32)  # [h, t, c2, h', m]
    for h in (0, 1):
        pad[h, :, :, h, :] = flat[2 * np.arange(8) + h].transpose(1, 0, 2)
    return np.ascontiguousarray(pad.reshape(128, 1024).astype(BF16))


def _unpack_out(out_pack: np.ndarray, out_full: np.ndarray, r: int) -> None:
    # out_pack[64h+m, 128c2+n] = out[4r+b(c), n, a(c)*64+m], c = 2*c2 + h
    o = out_pack.astype(F32).reshape(2, TS, 8, NF)  # [h, m, c2, n]
    for c2 in range(8):
        for h in (0, 1):
            c = 2 * c2 + h
            b, a = divmod(c, 4)
            out_full[4 * r + b, :, a * TS:(a + 1) * TS] = o[h, :, c2, :].T


# ---------------------------------------------------------------------------
# Device program
# ---------------------------------------------------------------------------

_prog_cache = {}


def _emit_body(nc, tc, pool, ps_pool, f32, bf16, xkid_d, wt01_d, wt23_d,
               x0_d, bias_d, out_d, n_warm=2):
    # ---- input DMAs ----
    # SDMA drains transfers in global issue order, so issue everything on
    # the sync ring in need-order: xk+identity (gates step-1 rhs and the
    # shuffle), weights (pace step 1), x0 (step 2 only), bias last.
    xkid_s = pool.tile([128, 1152], bf16, tag="xkid")
    nc.sync.dma_start(xkid_s[:], xkid_d.ap())
    wt01_s = pool.tile([128, 2048], bf16, tag="wt01")
    nc.sync.dma_start(wt01_s[:], wt01_d.ap())
    wt23_s = pool.tile([128, 2048], bf16, tag="wt23")
    nc.sync.dma_start(wt23_s[:], wt23_d.ap())
    x0p_s = pool.tile([128, 1024], bf16, tag="x0p")
    nc.sync.dma_start(x0p_s[:], x0_d.ap())
    bias_s = pool.tile([1, 512], bf16, tag="bias")
    nc.sync.dma_start(bias_s[:], bias_d.ap())

    xk_pad = xkid_s[:, 0:1024]
    ident = xkid_s[:, 1024:1152]

    # small constants (gpsimd, early, off the critical path)
    warm_s = pool.tile([128, 512], bf16, tag="warm")
    nc.gpsimd.memset(warm_s[:], 0.0)
    ones_s = pool.tile([128, 128], bf16, tag="ones")
    nc.gpsimd.memset(ones_s[:], 1.0)

    # PE warm-up (HAM clock ramp) on the zeroed tile
    ps_w = ps_pool.tile([128, 512], f32, tag="warm_ps")
    for _ in range(n_warm):
        nc.tensor.matmul(ps_w[:, :], warm_s[:, 0:128], warm_s[:, :],
                         start=True, stop=True)

    def wt_cols(p):  # lhsT tile [128, 128] for pair p
        if p < 16:
            return wt01_s[:, 128 * p:128 * (p + 1)]
        return wt23_s[:, 128 * (p - 16):128 * (p - 15)]

    # pre-allocate all PSUM tiles (4 tags x bufs=2 = all 8 banks, no reuse)
    ps1, ps_t, ps2 = [], [], []
    for i in range(2):
        ps1_i = ps_pool.tile([128, 512], f32, tag="s1")
        ps_t_i = ps_pool.tile([128, 512], f32, tag="t2")
        ps2_i = ps_pool.tile([128, 512], f32, tag="s2")
        ps1.append(ps1_i)
        ps_t.append(ps_t_i)
        ps2.append(ps2_i)

    # ---- step 1: W2 = wt_p.T @ xk_pad_p (contract (q,y)) ----
    # psum cols 32p+16q'+2c2+h; the scatter copy (split by h across
    # DVE/ACT) lands W2n[n; 128c2 + 64h + (2p+q')] = [n; 128c2+64h+t].
    w2n_s = pool.tile([128, 1024], bf16, tag="w2n")

    def emit_s1(u):
        for p in range(16 * u, 16 * u + 16):
            nc.tensor.matmul(
                ps1[u][:, 32 * (p % 16):32 * (p % 16 + 1)],
                wt_cols(p),
                xk_pad[:, 32 * p:32 * (p + 1)],
                start=True, stop=True,
            )
        # single full-width scatter copy on DVE (both h halves at once)
        src = ps1[u][:, :].rearrange(
            "z (tl c2 h) -> z c2 h tl", tl=32, c2=8)
        dst = w2n_s[:].rearrange(
            "z (c2 hh uu tl) -> z c2 hh uu tl",
            c2=8, hh=2, uu=2, tl=32)[:, :, :, u, :]
        nc.vector.tensor_copy(dst, src)

    emit_s1(0)

    # bias pre-load of the step-2 PSUM banks: psum[:, (f,n)] = bias[n]
    # (K=1 matmul, ones-row (x) bias-row; runs in the wt-DMA-wait shadow)
    for u in range(2):
        nc.tensor.matmul(ps2[u][:, :], ones_s[0:1, 0:128], bias_s[0:1, :],
                         start=True, stop=False)

    emit_s1(1)

    # ---- shuffle: w2r[64h+t; 128c2+n] via 8 identity matmuls ----
    w2r_s = pool.tile([128, 1024], bf16, tag="w2r")
    for v in range(2):
        for c2 in range(4 * v, 4 * v + 4):
            nc.tensor.matmul(
                ps_t[v][:, 128 * (c2 % 4):128 * (c2 % 4 + 1)],
                w2n_s[:, 128 * c2:128 * (c2 + 1)],
                ident,
                start=True, stop=True,
            )
        cols = slice(512 * v, 512 * (v + 1))
        if v == 0:
            nc.scalar.copy(w2r_s[:, cols], ps_t[v][:, :])
        else:
            nc.vector.tensor_copy(w2r_s[:, cols], ps_t[v][:, :])

    # ---- step 2: psum(bias) += x0pad.T @ w2r (contract (64h+t)) ----
    out_s = pool.tile([128, 1024], bf16, tag="out")
    for u in range(2):
        for c2 in range(4 * u, 4 * u + 4):
            nc.tensor.matmul(
                ps2[u][:, 128 * (c2 % 4):128 * (c2 % 4 + 1)],
                x0p_s[:, 128 * c2:128 * (c2 + 1)],
                w2r_s[:, 128 * c2:128 * (c2 + 1)],
                start=False, stop=True,
            )
        cols = slice(512 * u, 512 * (u + 1))
        if u == 0:
            nc.vector.tensor_copy(out_s[:, cols], ps2[u][:, :])
        else:
            nc.scalar.copy(out_s[:, cols], ps2[u][:, :])
        eng = nc.sync if u == 0 else nc.scalar
        eng.dma_start(out_d.ap()[:, 512 * u:512 * (u + 1)],
                      out_s[:, 512 * u:512 * (u + 1)])


def _build_program(version=14):
    if version in _prog_cache:
        return _prog_cache[version]

    from contextlib import ExitStack

    import concourse.bacc as bacc
    import concourse.mybir as mybir
    import concourse.tile as tile

    f32 = mybir.dt.float32
    bf16 = mybir.dt.bfloat16
    nc = bacc.Bacc("TRN2", target_bir_lowering=False, debug=False)

    xkid_d = nc.dram_tensor("xkid_pack", [128, 1152], bf16,
                            kind="ExternalInput")
    wt01_d = nc.dram_tensor("wt01_pack", [128, 2048], bf16,
                            kind="ExternalInput")
    wt23_d = nc.dram_tensor("wt23_pack", [128, 2048], bf16,
                            kind="ExternalInput")
    x0_d = nc.dram_tensor("x0_pack", [128, 1024], bf16, kind="ExternalInput")
    bias_d = nc.dram_tensor("bias_pack", [1, 512], bf16, kind="ExternalInput")
    out_d = nc.dram_tensor("out_pack", [128, 1024], bf16,
                           kind="ExternalOutput")

    with tile.TileContext(nc) as tc, ExitStack() as ctx:
        pool = ctx.enter_context(tc.tile_pool(name="io", bufs=1))
        ps_pool = ctx.enter_context(tc.tile_pool(name="ps", bufs=2, space="PSUM"))
        _emit_body(nc, tc, pool, ps_pool, f32, bf16, xkid_d, wt01_d, wt23_d,
                   x0_d, bias_d, out_d)

    nc.compile()
    _prog_cache[version] = nc
    return nc


def pack_core_inputs(x_0, x_k, conv_w, conv_b, version=14):
    """Returns in_maps (list of 8 dicts) for run_bass_kernel_spmd."""
    wt = _pack_wt(np.asarray(conv_w, dtype=F32))
    wt01 = np.ascontiguousarray(wt[:, 0:2048])
    wt23 = np.ascontiguousarray(wt[:, 2048:4096])
    bias4 = np.ascontiguousarray(
        np.tile(np.asarray(conv_b, dtype=F32), 4)[None, :].astype(BF16))
    ident = np.eye(128, dtype=BF16)
    x0 = np.asarray(x_0, dtype=F32)
    xk = np.asarray(x_k, dtype=F32)
    in_maps = []
    for r in range(NCORES):
        xkid = np.concatenate(
            [_pack_xk_padded(xk[B * r:B * (r + 1)]), ident], axis=1)
        in_maps.append({
            "xkid_pack": np.ascontiguousarray(xkid),
            "wt01_pack": wt01,
            "wt23_pack": wt23,
            "x0_pack": _pack_x0_padded(x0[B * r:B * (r + 1)]),
            "bias_pack": bias4,
        })
    return in_maps


VERSION = 14


def kernel(x_0, x_k, conv_w, conv_b):
    from concourse.bass_utils import run_bass_kernel_spmd

    nc = _build_program(VERSION)
    in_maps = pack_core_inputs(x_0, x_k, conv_w, conv_b, version=VERSION)
    res = run_bass_kernel_spmd(nc, in_maps, core_ids=list(range(NCORES)))
    out = np.empty((BS, NF, F), dtype=F32)
    for r in range(NCORES):
        _unpack_out(res.results[r]["out_pack"], out, r)
    return out


# ---------------------------------------------------------------------------
# numpy model of the packed device program (for testing the packing logic)
# ---------------------------------------------------------------------------

def _numpy_model(x_0, x_k, conv_w, conv_b):
    out = np.empty((BS, NF, F), dtype=F32)
    in_maps = pack_core_inputs(x_0, x_k, conv_w, conv_b)
    for r in range(NCORES):
        m = in_maps[r]
        xk_pad = m["xkid_pack"][:, :1024].astype(F32)
        wt = np.concatenate([m["wt01_pack"], m["wt23_pack"]],
                            axis=1).astype(F32)
        x0l = m["x0_pack"].astype(F32)
        bias4 = m["bias_pack"].astype(F32)  # [1, 512] = bias tiled 4x
        # step 1 + scatter copy: W2n[n; 128c2 + 64h + t], t = 2p + q'
        w2n = np.zeros((128, 8, 2, 64), dtype=F32)  # [n, c2, h, t]
        for p in range(32):
            blk = (wt[:, 128 * p:128 * (p + 1)].T
                   @ xk_pad[:, 32 * p:32 * (p + 1)])  # [n, (q',c2,h)]
            blk = blk.reshape(128, 2, 8, 2)
            for qp in range(2):
                w2n[:, :, :, 2 * p + qp] = blk[:, qp].transpose(0, 1, 2)
        w2n = w2n.reshape(128, 1024).astype(BF16).astype(F32)
        # shuffle
        w2r = np.zeros((128, 1024), dtype=F32)
        for c2 in range(8):
            w2r[:, 128 * c2:128 * (c2 + 1)] = (
                w2n[:, 128 * c2:128 * (c2 + 1)].T)
        w2r = w2r.astype(BF16).astype(F32)
        # step 2 (psum pre-loaded with bias via ones (x) bias4)
        out_pack = np.empty((128, 1024), dtype=F32)
        for u in range(2):
            out_pack[:, 512 * u:512 * (u + 1)] = bias4
        for c2 in range(8):
            out_pack[:, 128 * c2:128 * (c2 + 1)] += (
                x0l[:, 128 * c2:128 * (c2 + 1)].T
                @ w2r[:, 128 * c2:128 * (c2 + 1)]
            )
        _unpack_out(out_pack.astype(BF16), out, r)
    return out
